# revision 8
# baseline (speedup 1.0000x reference)
"""Trainium2 Bass kernel for nn_AttentionBlock: 8-core data-parallel over batch.

Reference computation (per batch b):
  cx = X[b] @ Wx^T               [K,R]   (K=49 regions, R=49, H=1024)
  ch = h_t[b] @ Wh^T             [T,R]   (T=128)
  z[t,k] = sum_r Wa[r] * tanh(cx[k,r] + ch[t,r])
  alpha = softmax_k(z)           [T,K]
  out[b] = alpha @ X[b]          [T,H]

Sharding: data-parallel across batch B=128 on 8 cores (16 batches each);
weights replicated. No collectives.

Layout strategy per batch:
  - load h_t[b] [T,H], X[b] [K,H] naturally; transpose 128-col strips on PE
    (is_transpose matmuls) into PSUM, copy back to SBUF -> hT [h,t], xT [h,k]
  - chT[r,t] / cxT[r,k] via matmuls contracting h (weights WT stationary)
  - replicate r-dim twice (98 partitions): S[98, 25, 128] = chT2 + cxT2
    (DVE broadcast add), tanh on ScalarE
  - z[t, 2q:2q+2] = S[:,q,:].T @ Wa2 (block-diag Wa) -> z [T, 49]
  - free-axis softmax with fused exp+accum; alpha^T via PE transpose
  - out[b] = alphaT.T @ X[b] from natural X; copy PSUM->SBUF; DMA out
"""

import sys

sys.path.insert(0, "/opt/trn_rl_repo")

import numpy as np

import concourse.bass as bass
import concourse.bacc as bacc
import concourse.tile as tile
from concourse import mybir
from concourse.bass_utils import run_bass_kernel_spmd
from concourse.masks import make_identity

B, T, K, H = 128, 128, 49, 1024
R = 49
NCORES = 8
BL = B // NCORES  # batches per core
HT = H // 128  # h tiles
NPAIR = (K + 1) // 2  # 25 k-pairs (last pair half-garbage, ignored)
PB = 64  # partition offset of the second r-block (must be mult of 32)
PT = PB + R  # 113 partitions used by the packed S / Wa2
F32 = mybir.dt.float32

_CACHE = {}


def _ap(base, off, dims):
    """Custom access pattern on the tensor underlying `base` (an AP)."""
    return bass.AP(tensor=base.tensor, offset=base.offset + off, ap=dims)


def _ap3(tile_ap, inner):
    """3D view [P, n, inner] of a tile whose free size is n*inner."""
    p_step, p_cnt = tile_ap.ap[0]
    free = tile_ap.free_size()
    return bass.AP(
        tensor=tile_ap.tensor,
        offset=tile_ap.offset,
        ap=[[p_step, p_cnt], [inner, free // inner], [1, inner]],
    )


def build():
    nc = bacc.Bacc("TRN2", target_bir_lowering=False, debug=False, num_devices=NCORES)

    X_d = nc.dram_tensor("X", [BL, K, H], F32, kind="ExternalInput").ap()
    ht_d = nc.dram_tensor("h_t", [BL, T, H], F32, kind="ExternalInput").ap()
    Wx_d = nc.dram_tensor("Wx", [R, H], F32, kind="ExternalInput").ap()
    Wh_d = nc.dram_tensor("Wh", [R, H], F32, kind="ExternalInput").ap()
    Wa_d = nc.dram_tensor("Wa", [1, R], F32, kind="ExternalInput").ap()
    out_d = nc.dram_tensor("out", [BL, T, H], F32, kind="ExternalOutput").ap()

    with tile.TileContext(nc) as tc:
        with (
            tc.tile_pool(name="consts", bufs=1) as consts,
            tc.tile_pool(name="hin", bufs=2) as hin_pool,
            tc.tile_pool(name="xin", bufs=2) as xin_pool,
            tc.tile_pool(name="hTp", bufs=2) as hT_pool,
            tc.tile_pool(name="xTp", bufs=2) as xT_pool,
            tc.tile_pool(name="wk", bufs=2) as wk,
            tc.tile_pool(name="sm", bufs=2) as sm,
            tc.tile_pool(name="pcc", bufs=1, space="PSUM") as pcc,
            tc.tile_pool(name="pat", bufs=1, space="PSUM") as pat,
            tc.tile_pool(name="ptp", bufs=1, space="PSUM") as ptp,
            tc.tile_pool(name="psZ", bufs=2, space="PSUM") as psZ,
            tc.tile_pool(name="psO", bufs=1, space="PSUM") as psO,
        ):
            # ---- identity for PE transposes ----
            ident = consts.tile([128, 128], F32)
            make_identity(nc, ident[:])

            # ---- weights: load natural, PE-transpose to WT [h(128p), j, r] ----
            def load_wt(w_dram, tag):
                wn = consts.tile([R, H], F32, tag="wnat")
                nc.sync.dma_start(out=wn[:], in_=_ap(w_dram, 0, [[H, R], [1, H]]))
                wt = consts.tile([128, HT * R], F32, tag=tag)
                tp = ptp.tile([128, HT * R], F32, tag="tp")
                for j in range(HT):
                    nc.tensor.transpose(
                        tp[:, j * R : (j + 1) * R],
                        wn[:, j * 128 : (j + 1) * 128],
                        ident[0:R, 0:R],
                    )
                nc.vector.tensor_copy(wt[:], tp[:])
                return wt

            WhT = load_wt(Wh_d, "WhT")  # [128, 8*49]; j-th tile at cols j*49
            WxT = load_wt(Wx_d, "WxT")

            # Wa2: [128, 2] block-diagonal: rows 0:49 col0 = Wa, rows 64:113 col1 = Wa
            # (compute-engine partition accesses must start at 0/32/64/96, so the
            #  second block sits at partition 64; rows 49:64 stay zero)
            Wa2 = consts.tile([128, 2], F32)
            nc.vector.memset(Wa2[:], 0.0)
            nc.sync.dma_start(out=Wa2[0:R, 0:1], in_=_ap(Wa_d, 0, [[1, R], [1, 1]]))
            nc.sync.dma_start(out=Wa2[PB : PB + R, 1:2], in_=_ap(Wa_d, 0, [[1, R], [1, 1]]))

            for b in range(BL):
                # ---- natural loads ----
                hn = hin_pool.tile([T, H], F32, tag="hn")
                nc.sync.dma_start(
                    out=hn[:], in_=_ap(ht_d, b * T * H, [[H, T], [1, H]])
                )
                xn = xin_pool.tile([K, H], F32, tag="xn")
                nc.sync.dma_start(
                    out=xn[:], in_=_ap(X_d, b * K * H, [[H, K], [1, H]])
                )

                # ---- transpose h_t[b] on PE: hT [128, j, t] (2 rounds of 4) ----
                hT = hT_pool.tile([128, HT, T], F32, tag="hT")
                for rnd in range(2):
                    tp = ptp.tile([128, 512], F32, tag="tp")
                    for jj in range(4):
                        j = 4 * rnd + jj
                        nc.tensor.transpose(
                            tp[:, jj * 128 : (jj + 1) * 128],
                            hn[:, j * 128 : (j + 1) * 128],
                            ident[:],
                        )
                    nc.vector.tensor_copy(
                        hT[:, 4 * rnd : 4 * rnd + 4, :], tp[:]
                    )

                # ---- transpose X[b] on PE: xT [128, j, k] ----
                xT = xT_pool.tile([128, HT, K], F32, tag="xT")
                tpx = ptp.tile([128, HT * K], F32, tag="tp")
                for j in range(HT):
                    nc.tensor.transpose(
                        tpx[:, j * K : (j + 1) * K],
                        xn[:, j * 128 : (j + 1) * 128],
                        ident[0:K, 0:K],
                    )
                nc.vector.tensor_copy(xT[:], _ap3(tpx[:], K))

                # ---- chT = Wh @ h_t[b]^T : [49, 128]; cxT = Wx @ X^T : [49, 49] ----
                cc = pcc.tile([R, T + K], F32, tag="cc")
                chT = cc[:, 0:T]
                cxT = cc[:, T : T + K]
                for j in range(HT):
                    nc.tensor.matmul(
                        chT, WhT[:, j * R : (j + 1) * R], hT[:, j, :],
                        start=(j == 0), stop=(j == HT - 1),
                    )
                for j in range(HT):
                    nc.tensor.matmul(
                        cxT, WxT[:, j * R : (j + 1) * R], xT[:, j, :],
                        start=(j == 0), stop=(j == HT - 1),
                    )

                # replicate chT into partitions 0:49 and 64:113; zero the gap rows
                chT2 = wk.tile([128, T], F32, tag="chT2")
                nc.vector.memset(chT2[32:PB, :], 0.0)
                nc.vector.tensor_copy(chT2[0:R, :], chT)
                nc.vector.tensor_copy(chT2[PB : PB + R, :], chT)

                # cxT2[128, 25]: rows 0:49 = even k columns, rows 64:113 = odd
                cxT2 = wk.tile([128, NPAIR], F32, tag="cxT2")
                nc.vector.memset(cxT2[:], 0.0)
                st = cxT.ap[-1][0]
                nc.vector.tensor_copy(
                    cxT2[0:R, 0:NPAIR], _ap(cxT, 0, [cxT.ap[0], [2 * st, NPAIR]])
                )
                nc.vector.tensor_copy(
                    cxT2[PB : PB + R, 0 : NPAIR - 1],
                    _ap(cxT, st, [cxT.ap[0], [2 * st, NPAIR - 1]]),
                )

                # ---- S = tanh(chT2 (bcast over q) + cxT2 (bcast over t)) ----
                S = sm.tile([128, NPAIR, T], F32, tag="S")
                c2 = chT2[:]
                x2 = cxT2[:]
                nc.vector.tensor_add(
                    S[0:PT, :, :],
                    _ap(c2, 0, [[c2.ap[0][0], PT], [0, NPAIR], c2.ap[-1]]),
                    _ap(x2, 0, [[x2.ap[0][0], PT], x2.ap[-1], [0, T]]),
                )
                nc.scalar.activation(
                    S[0:PT, :, :], S[0:PT, :, :], mybir.ActivationFunctionType.Tanh
                )

                # ---- z[t, k]: 25 matmuls, pair q -> columns (2q, 2q+1) ----
                z = psZ.tile([T, 2 * NPAIR], F32, tag="z")
                for q in range(NPAIR):
                    nc.tensor.matmul(
                        z[:, 2 * q : 2 * q + 2], S[0:PT, q, :], Wa2[0:PT, :],
                        start=True, stop=True,
                    )

                # ---- softmax over k (free axis), K=49 valid columns ----
                zmax = sm.tile([T, 1], F32, tag="zmax")
                nc.vector.reduce_max(zmax[:], z[:, 0:K], axis=mybir.AxisListType.X)
                zmaxn = sm.tile([T, 1], F32, tag="zmaxn")
                nc.vector.tensor_scalar_mul(zmaxn[:], zmax[:], -1.0)
                expz = sm.tile([T, K], F32, tag="expz")
                denom = sm.tile([T, 1], F32, tag="denom")
                nc.scalar.activation(
                    expz[:], z[:, 0:K], mybir.ActivationFunctionType.Exp,
                    bias=zmaxn[:], accum_out=denom[:],
                )
                rden = sm.tile([T, 1], F32, tag="rden")
                nc.vector.reciprocal(rden[:], denom[:])
                alpha = sm.tile([T, K], F32, tag="alpha")
                nc.vector.tensor_scalar_mul(alpha[:], expz[:], rden[:])

                # ---- alphaT via PE transpose ----
                alphaT_ps = pat.tile([K, T], F32, tag="alphaT")
                nc.tensor.transpose(alphaT_ps[:], alpha[:], ident[:])
                alphaT = sm.tile([K, T], F32, tag="alphaT_sb")
                nc.vector.tensor_copy(alphaT[:], alphaT_ps[:])

                # ---- out[b] = alpha @ X[b] : [128, 1024] ----
                ob = psO.tile([T, H], F32, tag="ob")
                for half in range(2):
                    nc.tensor.matmul(
                        ob[:, half * 512 : (half + 1) * 512],
                        alphaT[:],
                        xn[:, half * 512 : (half + 1) * 512],
                        start=True, stop=True,
                    )
                osb = sm.tile([T, H], F32, tag="osb")
                nc.vector.tensor_copy(osb[:], ob[:])
                nc.sync.dma_start(
                    out=_ap(out_d, b * T * H, [[H, T], [1, H]]), in_=osb[:]
                )

    nc.compile()
    return nc


def _get_nc():
    if "nc" not in _CACHE:
        _CACHE["nc"] = build()
    return _CACHE["nc"]


def kernel(X, h_t, Wx, Wh, Wa):
    nc = _get_nc()
    X = np.ascontiguousarray(X, dtype=np.float32)
    h_t = np.ascontiguousarray(h_t, dtype=np.float32)
    Wx = np.ascontiguousarray(Wx, dtype=np.float32)
    Wh = np.ascontiguousarray(Wh, dtype=np.float32)
    Wa = np.ascontiguousarray(Wa, dtype=np.float32)
    in_maps = [
        {
            "X": X[c * BL : (c + 1) * BL],
            "h_t": h_t[c * BL : (c + 1) * BL],
            "Wx": Wx,
            "Wh": Wh,
            "Wa": Wa,
        }
        for c in range(NCORES)
    ]
    res = run_bass_kernel_spmd(nc, in_maps, core_ids=list(range(NCORES)))
    return np.concatenate([res.results[c]["out"] for c in range(NCORES)], axis=0)


# revision 12
# speedup vs baseline: 1.9833x; 1.9833x over previous
"""Trainium2 Bass kernel for nn_AttentionBlock: 8-core data-parallel over batch.

Reference computation (per batch b):
  cx = X[b] @ Wx^T               [K,R]   (K=49 regions, R=49, H=1024)
  ch = h_t[b] @ Wh^T             [T,R]   (T=128)
  z[t,k] = sum_r Wa[r] * tanh(cx[k,r] + ch[t,r])
  alpha = softmax_k(z)           [T,K]
  out[b] = alpha @ X[b]          [T,H]

Sharding: data-parallel across batch B=128 on 8 cores (16 batches each);
weights replicated. No collectives.

v2 layout strategy per batch (all matmuls in bf16, fp32 PSUM accum):
  - load h_t[b]/X[b] naturally (f32), convert to bf16 on GpSimd,
    transpose via xbar DMA (SBUF->SBUF, 2-byte dtype) -> hTb [h,j,t], xTb [h,j,k]
  - chT[r,t] / cxT[r,k] via bf16 matmuls contracting h (WT stationary)
  - pack r twice (partitions 0:49 and 64:113): S[113, 25, 128] =
    tanh(chT2 + cxT2) via DVE broadcast add + one big ScalarE tanh
  - z[t, 2q:2q+2] = S[:,q,:].T @ Wa2 (block-diag Wa, zero rows kill the gap)
  - free-axis softmax with fused exp+accum; alpha^T via PE transpose
  - out[b] = alphaT.T @ X[b] (bf16); copy PSUM->SBUF; DMA out (f32)
"""

import sys

sys.path.insert(0, "/opt/trn_rl_repo")

import numpy as np

import concourse.bass as bass
import concourse.bacc as bacc
import concourse.tile as tile
from concourse import mybir
from concourse.bass_utils import run_bass_kernel_spmd
from concourse.masks import make_identity

B, T, K, H = 128, 128, 49, 1024
R = 49
NCORES = 8
BL = B // NCORES  # batches per core
HT = H // 128  # h tiles
NPAIR = (K + 1) // 2  # 25 k-pairs (last pair half-garbage, ignored)
PB = 64  # partition offset of the second r-block (must be mult of 32)
PT = PB + R  # 113 partitions used by the packed S / Wa2
KP = 64  # X partition count padded for xbar transpose (needs mult of 16)
F32 = mybir.dt.float32
BF16 = mybir.dt.bfloat16

_CACHE = {}


def _ap(base, off, dims):
    """Custom access pattern on the tensor underlying `base` (an AP)."""
    return bass.AP(tensor=base.tensor, offset=base.offset + off, ap=dims)


def build():
    nc = bacc.Bacc("TRN2", target_bir_lowering=False, debug=False, num_devices=NCORES)

    X_d = nc.dram_tensor("X", [BL, K, H], F32, kind="ExternalInput").ap()
    ht_d = nc.dram_tensor("h_t", [BL, T, H], F32, kind="ExternalInput").ap()
    Wx_d = nc.dram_tensor("Wx", [R, H], F32, kind="ExternalInput").ap()
    Wh_d = nc.dram_tensor("Wh", [R, H], F32, kind="ExternalInput").ap()
    Wa_d = nc.dram_tensor("Wa", [1, R], F32, kind="ExternalInput").ap()
    out_d = nc.dram_tensor("out", [BL, T, H], F32, kind="ExternalOutput").ap()

    with tile.TileContext(nc) as tc:
        with (
            tc.tile_pool(name="consts", bufs=1) as consts,
            tc.tile_pool(name="hin", bufs=2) as hin_pool,
            tc.tile_pool(name="xin", bufs=2) as xin_pool,
            tc.tile_pool(name="hbp", bufs=2) as hb_pool,
            tc.tile_pool(name="xbp", bufs=2) as xb_pool,
            tc.tile_pool(name="hTp", bufs=2) as hT_pool,
            tc.tile_pool(name="xTp", bufs=2) as xT_pool,
            tc.tile_pool(name="wk", bufs=2) as wk,
            tc.tile_pool(name="sm", bufs=2) as sm,
            tc.tile_pool(name="pcc", bufs=2, space="PSUM") as pcc,
            tc.tile_pool(name="pat", bufs=1, space="PSUM") as pat,
            tc.tile_pool(name="ptp", bufs=1, space="PSUM") as ptp,
            tc.tile_pool(name="psZ", bufs=2, space="PSUM") as psZ,
            tc.tile_pool(name="psO", bufs=1, space="PSUM") as psO,
        ):
            # ---- identities for PE transposes ----
            ident = consts.tile([128, 128], F32)
            make_identity(nc, ident[:])
            identb = consts.tile([128, 128], BF16)
            make_identity(nc, identb[:])

            # ---- weights: load natural f32, PE-transpose, store bf16 ----
            def load_wt(w_dram, tag):
                wn = consts.tile([R, H], F32, tag="wnat")
                nc.sync.dma_start(out=wn[:], in_=_ap(w_dram, 0, [[H, R], [1, H]]))
                wt = consts.tile([128, HT * R], BF16, tag=tag)
                tp = ptp.tile([128, HT * R], F32, tag="tp")
                for j in range(HT):
                    nc.tensor.transpose(
                        tp[:, j * R : (j + 1) * R],
                        wn[:, j * 128 : (j + 1) * 128],
                        ident[0:R, 0:R],
                    )
                nc.vector.tensor_copy(wt[:], tp[:])
                return wt

            WhT = load_wt(Wh_d, "WhT")  # [128, 8*49] bf16; j-th tile at cols j*49
            WxT = load_wt(Wx_d, "WxT")

            # Wa2: [128, 2] block-diag: rows 0:49 col0 = Wa, rows 64:113 col1 = Wa
            Wa2f = consts.tile([128, 2], F32)
            nc.vector.memset(Wa2f[:], 0.0)
            nc.sync.dma_start(out=Wa2f[0:R, 0:1], in_=_ap(Wa_d, 0, [[1, R], [1, 1]]))
            nc.sync.dma_start(
                out=Wa2f[PB : PB + R, 1:2], in_=_ap(Wa_d, 0, [[1, R], [1, 1]])
            )
            Wa2 = consts.tile([128, 2], BF16)
            nc.vector.tensor_copy(Wa2[:], Wa2f[:])

            for b in range(BL):
                # ---- natural loads (f32) ----
                hn = hin_pool.tile([T, H], F32, tag="hn")
                nc.sync.dma_start(
                    out=hn[:], in_=_ap(ht_d, b * T * H, [[H, T], [1, H]])
                )
                xn = xin_pool.tile([K, H], F32, tag="xn")
                nc.sync.dma_start(
                    out=xn[0:K, :], in_=_ap(X_d, b * K * H, [[H, K], [1, H]])
                )

                # ---- bf16 conversion on DVE ----
                hb = hb_pool.tile([T, H], BF16, tag="hb")
                nc.vector.tensor_copy(hb[:], hn[:])
                xb = xb_pool.tile([K, H], BF16, tag="xb")
                nc.vector.tensor_copy(xb[:], xn[0:K, :])

                # ---- PE transposes (bf16): hTb[p,j,t] = hb[t, j*128+p] ----
                hTb = hT_pool.tile([128, HT, T], BF16, tag="hTb")
                for rnd in range(2):
                    tp = ptp.tile([128, 512], BF16, tag="tp")
                    for jj in range(4):
                        j = 4 * rnd + jj
                        nc.tensor.transpose(
                            tp[:, jj * 128 : (jj + 1) * 128],
                            hb[:, j * 128 : (j + 1) * 128],
                            identb[:],
                        )
                    nc.vector.tensor_copy(hTb[:, 4 * rnd : 4 * rnd + 4, :], tp[:])
                KA = 50  # padded column stride (4-byte-aligned bf16 PSUM offsets)
                xTb = xT_pool.tile([128, HT, K], BF16, tag="xTb")
                tpx = ptp.tile([128, HT * KA], BF16, tag="tp")
                for j in range(HT):
                    nc.tensor.transpose(
                        tpx[:, j * KA : j * KA + K],
                        xb[:, j * 128 : (j + 1) * 128],
                        identb[0:K, 0:K],
                    )
                nc.vector.tensor_copy(
                    xTb[:], bass.AP(tensor=tpx[:].tensor, offset=tpx[:].offset,
                                    ap=[tpx[:].ap[0], [KA, HT], [1, K]])
                )

                # ---- chT = Wh @ h_t[b]^T : [49, 128]; cxT = Wx @ X^T : [49, 49] ----
                cc = pcc.tile([R, T + K], F32, tag="cc")
                chT = cc[:, 0:T]
                cxT = cc[:, T : T + K]
                for j in range(HT):
                    nc.tensor.matmul(
                        chT, WhT[:, j * R : (j + 1) * R], hTb[:, j, :],
                        start=(j == 0), stop=(j == HT - 1),
                    )
                for j in range(HT):
                    nc.tensor.matmul(
                        cxT, WxT[:, j * R : (j + 1) * R], xTb[:, j, :],
                        start=(j == 0), stop=(j == HT - 1),
                    )

                # replicate chT into partitions 0:49 and 64:113; zero the gap rows
                chT2 = wk.tile([128, T], BF16, tag="chT2")
                nc.vector.memset(chT2[32:PB, :], 0.0)
                nc.vector.tensor_copy(chT2[0:R, :], chT)
                nc.vector.tensor_copy(chT2[PB : PB + R, :], chT)

                # cxT2[128, 25]: rows 0:49 = even k columns, rows 64:113 = odd
                cxT2 = wk.tile([128, NPAIR], BF16, tag="cxT2")
                nc.vector.memset(cxT2[:], 0.0)
                st = cxT.ap[-1][0]
                nc.vector.tensor_copy(
                    cxT2[0:R, 0:NPAIR], _ap(cxT, 0, [cxT.ap[0], [2 * st, NPAIR]])
                )
                nc.vector.tensor_copy(
                    cxT2[PB : PB + R, 0 : NPAIR - 1],
                    _ap(cxT, st, [cxT.ap[0], [2 * st, NPAIR - 1]]),
                )

                # ---- S = tanh(chT2 (bcast over q) + cxT2 (bcast over t)), bf16 ----
                S = sm.tile([128, NPAIR, T], BF16, tag="S")
                c2 = chT2[:]
                x2 = cxT2[:]
                nc.vector.tensor_add(
                    S[0:PT, :, :],
                    _ap(c2, 0, [[c2.ap[0][0], PT], [0, NPAIR], c2.ap[-1]]),
                    _ap(x2, 0, [[x2.ap[0][0], PT], x2.ap[-1], [0, T]]),
                )
                nc.scalar.activation(
                    S[0:PT, :, :], S[0:PT, :, :], mybir.ActivationFunctionType.Tanh
                )

                # ---- z[t, k]: 25 bf16 matmuls, pair q -> columns (2q, 2q+1) ----
                z = psZ.tile([T, 2 * NPAIR], F32, tag="z")
                for q in range(NPAIR):
                    nc.tensor.matmul(
                        z[:, 2 * q : 2 * q + 2], S[0:PT, q, :], Wa2[0:PT, :],
                        start=True, stop=True,
                    )

                # ---- softmax over k (free axis), K=49 valid columns ----
                zmax = sm.tile([T, 1], F32, tag="zmax")
                nc.vector.reduce_max(zmax[:], z[:, 0:K], axis=mybir.AxisListType.X)
                zmaxn = sm.tile([T, 1], F32, tag="zmaxn")
                nc.vector.tensor_scalar_mul(zmaxn[:], zmax[:], -1.0)
                expz = sm.tile([T, K], BF16, tag="expz")
                denom = sm.tile([T, 1], F32, tag="denom")
                nc.scalar.activation(
                    expz[:], z[:, 0:K], mybir.ActivationFunctionType.Exp,
                    bias=zmaxn[:], accum_out=denom[:],
                )
                rden = sm.tile([T, 1], F32, tag="rden")
                nc.vector.reciprocal(rden[:], denom[:])
                alpha = sm.tile([T, K], BF16, tag="alpha")
                nc.vector.tensor_scalar_mul(alpha[:], expz[:], rden[:])

                # ---- alphaT via PE transpose (bf16) ----
                alphaT_ps = pat.tile([K, T], BF16, tag="alphaT")
                nc.tensor.transpose(alphaT_ps[:], alpha[:], identb[:])
                alphaT = sm.tile([K, T], BF16, tag="alphaT_sb")
                nc.vector.tensor_copy(alphaT[:], alphaT_ps[:])

                # ---- out[b] = alpha @ X[b] : [128, 1024] (bf16 x bf16 -> f32) ----
                ob = psO.tile([T, H], F32, tag="ob")
                for half in range(2):
                    nc.tensor.matmul(
                        ob[:, half * 512 : (half + 1) * 512],
                        alphaT[:],
                        xb[:, half * 512 : (half + 1) * 512],
                        start=True, stop=True,
                    )
                osb = sm.tile([T, H], F32, tag="osb")
                nc.vector.tensor_copy(osb[:], ob[:])
                nc.sync.dma_start(
                    out=_ap(out_d, b * T * H, [[H, T], [1, H]]), in_=osb[:]
                )

    nc.compile()
    return nc


def _get_nc():
    if "nc" not in _CACHE:
        _CACHE["nc"] = build()
    return _CACHE["nc"]


def kernel(X, h_t, Wx, Wh, Wa):
    nc = _get_nc()
    X = np.ascontiguousarray(X, dtype=np.float32)
    h_t = np.ascontiguousarray(h_t, dtype=np.float32)
    Wx = np.ascontiguousarray(Wx, dtype=np.float32)
    Wh = np.ascontiguousarray(Wh, dtype=np.float32)
    Wa = np.ascontiguousarray(Wa, dtype=np.float32)
    in_maps = [
        {
            "X": X[c * BL : (c + 1) * BL],
            "h_t": h_t[c * BL : (c + 1) * BL],
            "Wx": Wx,
            "Wh": Wh,
            "Wa": Wa,
        }
        for c in range(NCORES)
    ]
    res = run_bass_kernel_spmd(nc, in_maps, core_ids=list(range(NCORES)))
    return np.concatenate([res.results[c]["out"] for c in range(NCORES)], axis=0)


# revision 13
# speedup vs baseline: 2.0357x; 1.0264x over previous
"""Trainium2 Bass kernel for nn_AttentionBlock: 8-core data-parallel over batch.

Reference computation (per batch b):
  cx = X[b] @ Wx^T               [K,R]   (K=49 regions, R=49, H=1024)
  ch = h_t[b] @ Wh^T             [T,R]   (T=128)
  z[t,k] = sum_r Wa[r] * tanh(cx[k,r] + ch[t,r])
  alpha = softmax_k(z)           [T,K]
  out[b] = alpha @ X[b]          [T,H]

Sharding: data-parallel across batch B=128 on 8 cores (16 batches each);
weights replicated. No collectives.

v2 layout strategy per batch (all matmuls in bf16, fp32 PSUM accum):
  - load h_t[b]/X[b] naturally (f32), convert to bf16 on GpSimd,
    transpose via xbar DMA (SBUF->SBUF, 2-byte dtype) -> hTb [h,j,t], xTb [h,j,k]
  - chT[r,t] / cxT[r,k] via bf16 matmuls contracting h (WT stationary)
  - pack r twice (partitions 0:49 and 64:113): S[113, 25, 128] =
    tanh(chT2 + cxT2) via DVE broadcast add + one big ScalarE tanh
  - z[t, 2q:2q+2] = S[:,q,:].T @ Wa2 (block-diag Wa, zero rows kill the gap)
  - free-axis softmax with fused exp+accum; alpha^T via PE transpose
  - out[b] = alphaT.T @ X[b] (bf16); copy PSUM->SBUF; DMA out (f32)
"""

import sys

sys.path.insert(0, "/opt/trn_rl_repo")

import numpy as np

import concourse.bass as bass
import concourse.bacc as bacc
import concourse.tile as tile
from concourse import mybir
from concourse.bass_utils import run_bass_kernel_spmd
from concourse.masks import make_identity

B, T, K, H = 128, 128, 49, 1024
R = 49
NCORES = 8
BL = B // NCORES  # batches per core
HT = H // 128  # h tiles
NPAIR = (K + 1) // 2  # 25 k-pairs (last pair half-garbage, ignored)
PB = 64  # partition offset of the second r-block (must be mult of 32)
PT = PB + R  # 113 partitions used by the packed S / Wa2
KP = 64  # X partition count padded for xbar transpose (needs mult of 16)
F32 = mybir.dt.float32
BF16 = mybir.dt.bfloat16

_CACHE = {}


def _ap(base, off, dims):
    """Custom access pattern on the tensor underlying `base` (an AP)."""
    return bass.AP(tensor=base.tensor, offset=base.offset + off, ap=dims)


def build():
    nc = bacc.Bacc("TRN2", target_bir_lowering=False, debug=False, num_devices=NCORES)

    X_d = nc.dram_tensor("X", [BL, K, H], F32, kind="ExternalInput").ap()
    ht_d = nc.dram_tensor("h_t", [BL, T, H], F32, kind="ExternalInput").ap()
    Wx_d = nc.dram_tensor("Wx", [R, H], F32, kind="ExternalInput").ap()
    Wh_d = nc.dram_tensor("Wh", [R, H], F32, kind="ExternalInput").ap()
    Wa_d = nc.dram_tensor("Wa", [1, R], F32, kind="ExternalInput").ap()
    out_d = nc.dram_tensor("out", [BL, T, H], F32, kind="ExternalOutput").ap()

    with tile.TileContext(nc) as tc:
        with (
            tc.tile_pool(name="consts", bufs=1) as consts,
            tc.tile_pool(name="hin", bufs=2) as hin_pool,
            tc.tile_pool(name="xin", bufs=2) as xin_pool,
            tc.tile_pool(name="hbp", bufs=2) as hb_pool,
            tc.tile_pool(name="xbp", bufs=2) as xb_pool,
            tc.tile_pool(name="hTp", bufs=2) as hT_pool,
            tc.tile_pool(name="xTp", bufs=2) as xT_pool,
            tc.tile_pool(name="wk", bufs=2) as wk,
            tc.tile_pool(name="sm", bufs=2) as sm,
            tc.tile_pool(name="pcc", bufs=2, space="PSUM") as pcc,
            tc.tile_pool(name="pat", bufs=1, space="PSUM") as pat,
            tc.tile_pool(name="ptp", bufs=1, space="PSUM") as ptp,
            tc.tile_pool(name="psZ", bufs=2, space="PSUM") as psZ,
            tc.tile_pool(name="psO", bufs=1, space="PSUM") as psO,
        ):
            # ---- identities for PE transposes ----
            ident = consts.tile([128, 128], F32)
            make_identity(nc, ident[:])
            identb = consts.tile([128, 128], BF16)
            make_identity(nc, identb[:])

            # ---- weights: load natural f32, PE-transpose, store bf16 ----
            def load_wt(w_dram, tag):
                wn = consts.tile([R, H], F32, tag="wnat")
                nc.sync.dma_start(out=wn[:], in_=_ap(w_dram, 0, [[H, R], [1, H]]))
                wt = consts.tile([128, HT * R], BF16, tag=tag)
                tp = ptp.tile([128, HT * R], F32, tag="tp")
                for j in range(HT):
                    nc.tensor.transpose(
                        tp[:, j * R : (j + 1) * R],
                        wn[:, j * 128 : (j + 1) * 128],
                        ident[0:R, 0:R],
                    )
                nc.vector.tensor_copy(wt[:], tp[:])
                return wt

            WhT = load_wt(Wh_d, "WhT")  # [128, 8*49] bf16; j-th tile at cols j*49
            WxT = load_wt(Wx_d, "WxT")

            # Wa2: [128, 2] block-diag: rows 0:49 col0 = Wa, rows 64:113 col1 = Wa
            Wa2f = consts.tile([128, 2], F32)
            nc.vector.memset(Wa2f[:], 0.0)
            nc.sync.dma_start(out=Wa2f[0:R, 0:1], in_=_ap(Wa_d, 0, [[1, R], [1, 1]]))
            nc.sync.dma_start(
                out=Wa2f[PB : PB + R, 1:2], in_=_ap(Wa_d, 0, [[1, R], [1, 1]])
            )
            Wa2 = consts.tile([128, 2], BF16)
            nc.vector.tensor_copy(Wa2[:], Wa2f[:])

            for b in range(BL):
                # ---- natural loads (f32) ----
                hn = hin_pool.tile([T, H], F32, tag="hn")
                nc.sync.dma_start(
                    out=hn[:], in_=_ap(ht_d, b * T * H, [[H, T], [1, H]])
                )
                xn = xin_pool.tile([K, H], F32, tag="xn")
                nc.sync.dma_start(
                    out=xn[0:K, :], in_=_ap(X_d, b * K * H, [[H, K], [1, H]])
                )

                # ---- bf16 conversion on DVE ----
                hb = hb_pool.tile([T, H], BF16, tag="hb")
                nc.scalar.copy(hb[:], hn[:])
                xb = xb_pool.tile([K, H], BF16, tag="xb")
                nc.vector.tensor_copy(xb[:], xn[0:K, :])

                # ---- PE transposes (bf16): hTb[p,j,t] = hb[t, j*128+p] ----
                hTb = hT_pool.tile([128, HT, T], BF16, tag="hTb")
                for rnd in range(2):
                    tp = ptp.tile([128, 512], BF16, tag="tp")
                    for jj in range(4):
                        j = 4 * rnd + jj
                        nc.tensor.transpose(
                            tp[:, jj * 128 : (jj + 1) * 128],
                            hb[:, j * 128 : (j + 1) * 128],
                            identb[:],
                        )
                    nc.vector.tensor_copy(hTb[:, 4 * rnd : 4 * rnd + 4, :], tp[:])
                KA = 50  # padded column stride (4-byte-aligned bf16 PSUM offsets)
                xTb = xT_pool.tile([128, HT, K], BF16, tag="xTb")
                tpx = ptp.tile([128, HT * KA], BF16, tag="tp")
                for j in range(HT):
                    nc.tensor.transpose(
                        tpx[:, j * KA : j * KA + K],
                        xb[:, j * 128 : (j + 1) * 128],
                        identb[0:K, 0:K],
                    )
                nc.vector.tensor_copy(
                    xTb[:], bass.AP(tensor=tpx[:].tensor, offset=tpx[:].offset,
                                    ap=[tpx[:].ap[0], [KA, HT], [1, K]])
                )

                # ---- chT = Wh @ h_t[b]^T : [49, 128]; cxT = Wx @ X^T : [49, 49] ----
                cc = pcc.tile([R, T + K], F32, tag="cc")
                chT = cc[:, 0:T]
                cxT = cc[:, T : T + K]
                for j in range(HT):
                    nc.tensor.matmul(
                        chT, WhT[:, j * R : (j + 1) * R], hTb[:, j, :],
                        start=(j == 0), stop=(j == HT - 1),
                    )
                for j in range(HT):
                    nc.tensor.matmul(
                        cxT, WxT[:, j * R : (j + 1) * R], xTb[:, j, :],
                        start=(j == 0), stop=(j == HT - 1),
                    )

                # replicate chT into partitions 0:49 and 64:113; zero the gap rows
                chT2 = wk.tile([128, T], BF16, tag="chT2")
                nc.vector.memset(chT2[32:PB, :], 0.0)
                nc.vector.tensor_copy(chT2[0:R, :], chT)
                nc.vector.tensor_copy(chT2[PB : PB + R, :], chT)

                # cxT2[128, 25]: rows 0:49 = even k columns, rows 64:113 = odd
                cxT2 = wk.tile([128, NPAIR], BF16, tag="cxT2")
                nc.vector.memset(cxT2[:], 0.0)
                st = cxT.ap[-1][0]
                nc.vector.tensor_copy(
                    cxT2[0:R, 0:NPAIR], _ap(cxT, 0, [cxT.ap[0], [2 * st, NPAIR]])
                )
                nc.vector.tensor_copy(
                    cxT2[PB : PB + R, 0 : NPAIR - 1],
                    _ap(cxT, st, [cxT.ap[0], [2 * st, NPAIR - 1]]),
                )

                # ---- S = tanh(chT2 (bcast over q) + cxT2 (bcast over t)), bf16 ----
                S = sm.tile([128, NPAIR, T], BF16, tag="S")
                c2 = chT2[:]
                x2 = cxT2[:]
                nc.vector.tensor_add(
                    S[0:PT, :, :],
                    _ap(c2, 0, [[c2.ap[0][0], PT], [0, NPAIR], c2.ap[-1]]),
                    _ap(x2, 0, [[x2.ap[0][0], PT], x2.ap[-1], [0, T]]),
                )
                nc.scalar.activation(
                    S[0:PT, :, :], S[0:PT, :, :], mybir.ActivationFunctionType.Tanh
                )

                # ---- z[t, k]: 25 bf16 matmuls, pair q -> columns (2q, 2q+1) ----
                z = psZ.tile([T, 2 * NPAIR], F32, tag="z")
                for q in range(NPAIR):
                    nc.tensor.matmul(
                        z[:, 2 * q : 2 * q + 2], S[0:PT, q, :], Wa2[0:PT, :],
                        start=True, stop=True,
                    )

                # ---- softmax over k (free axis), K=49 valid columns ----
                zmax = sm.tile([T, 1], F32, tag="zmax")
                nc.vector.reduce_max(zmax[:], z[:, 0:K], axis=mybir.AxisListType.X)
                zmaxn = sm.tile([T, 1], F32, tag="zmaxn")
                nc.vector.tensor_scalar_mul(zmaxn[:], zmax[:], -1.0)
                expz = sm.tile([T, K], BF16, tag="expz")
                denom = sm.tile([T, 1], F32, tag="denom")
                nc.scalar.activation(
                    expz[:], z[:, 0:K], mybir.ActivationFunctionType.Exp,
                    bias=zmaxn[:], accum_out=denom[:],
                )
                rden = sm.tile([T, 1], F32, tag="rden")
                nc.vector.reciprocal(rden[:], denom[:])
                alpha = sm.tile([T, K], BF16, tag="alpha")
                nc.vector.tensor_scalar_mul(alpha[:], expz[:], rden[:])

                # ---- alphaT via PE transpose (bf16) ----
                alphaT_ps = pat.tile([K, T], BF16, tag="alphaT")
                nc.tensor.transpose(alphaT_ps[:], alpha[:], identb[:])
                alphaT = sm.tile([K, T], BF16, tag="alphaT_sb")
                nc.vector.tensor_copy(alphaT[:], alphaT_ps[:])

                # ---- out[b] = alpha @ X[b] : [128, 1024] (bf16 x bf16 -> f32) ----
                ob = psO.tile([T, H], F32, tag="ob")
                for half in range(2):
                    nc.tensor.matmul(
                        ob[:, half * 512 : (half + 1) * 512],
                        alphaT[:],
                        xb[:, half * 512 : (half + 1) * 512],
                        start=True, stop=True,
                    )
                osb = sm.tile([T, H], F32, tag="osb")
                nc.vector.tensor_copy(osb[:, 0:512], ob[:, 0:512])
                nc.scalar.copy(osb[:, 512:1024], ob[:, 512:1024])
                nc.sync.dma_start(
                    out=_ap(out_d, b * T * H, [[H, T], [1, H]]), in_=osb[:]
                )

    nc.compile()
    return nc


def _get_nc():
    if "nc" not in _CACHE:
        _CACHE["nc"] = build()
    return _CACHE["nc"]


def kernel(X, h_t, Wx, Wh, Wa):
    nc = _get_nc()
    X = np.ascontiguousarray(X, dtype=np.float32)
    h_t = np.ascontiguousarray(h_t, dtype=np.float32)
    Wx = np.ascontiguousarray(Wx, dtype=np.float32)
    Wh = np.ascontiguousarray(Wh, dtype=np.float32)
    Wa = np.ascontiguousarray(Wa, dtype=np.float32)
    in_maps = [
        {
            "X": X[c * BL : (c + 1) * BL],
            "h_t": h_t[c * BL : (c + 1) * BL],
            "Wx": Wx,
            "Wh": Wh,
            "Wa": Wa,
        }
        for c in range(NCORES)
    ]
    res = run_bass_kernel_spmd(nc, in_maps, core_ids=list(range(NCORES)))
    return np.concatenate([res.results[c]["out"] for c in range(NCORES)], axis=0)


# revision 14
# speedup vs baseline: 2.4071x; 1.1825x over previous
"""Trainium2 Bass kernel for nn_AttentionBlock: 8-core data-parallel over batch.

Reference computation (per batch b):
  cx = X[b] @ Wx^T               [K,R]   (K=49 regions, R=49, H=1024)
  ch = h_t[b] @ Wh^T             [T,R]   (T=128)
  z[t,k] = sum_r Wa[r] * tanh(cx[k,r] + ch[t,r])
  alpha = softmax_k(z)           [T,K]
  out[b] = alpha @ X[b]          [T,H]

Sharding: data-parallel across batch B=128 on 8 cores (16 batches each);
weights replicated. No collectives.

v2 layout strategy per batch (all matmuls in bf16, fp32 PSUM accum):
  - load h_t[b]/X[b] naturally (f32), convert to bf16 on GpSimd,
    transpose via xbar DMA (SBUF->SBUF, 2-byte dtype) -> hTb [h,j,t], xTb [h,j,k]
  - chT[r,t] / cxT[r,k] via bf16 matmuls contracting h (WT stationary)
  - pack r twice (partitions 0:49 and 64:113): S[113, 25, 128] =
    tanh(chT2 + cxT2) via DVE broadcast add + one big ScalarE tanh
  - z[t, 2q:2q+2] = S[:,q,:].T @ Wa2 (block-diag Wa, zero rows kill the gap)
  - free-axis softmax with fused exp+accum; alpha^T via PE transpose
  - out[b] = alphaT.T @ X[b] (bf16); copy PSUM->SBUF; DMA out (f32)
"""

import sys

sys.path.insert(0, "/opt/trn_rl_repo")

import numpy as np

import concourse.bass as bass
import concourse.bacc as bacc
import concourse.tile as tile
from concourse import mybir
from concourse.bass_utils import run_bass_kernel_spmd
from concourse.masks import make_identity

B, T, K, H = 128, 128, 49, 1024
R = 49
NCORES = 8
BL = B // NCORES  # batches per core
HT = H // 128  # h tiles
NPAIR = (K + 1) // 2  # 25 k-pairs (last pair half-garbage, ignored)
PB = 64  # partition offset of the second r-block (must be mult of 32)
PT = PB + R  # 113 partitions used by the packed S / Wa2
KP = 64  # X partition count padded for xbar transpose (needs mult of 16)
F32 = mybir.dt.float32
BF16 = mybir.dt.bfloat16

_CACHE = {}


def _ap(base, off, dims):
    """Custom access pattern on the tensor underlying `base` (an AP)."""
    return bass.AP(tensor=base.tensor, offset=base.offset + off, ap=dims)


def build():
    nc = bacc.Bacc("TRN2", target_bir_lowering=False, debug=False, num_devices=NCORES)

    X_d = nc.dram_tensor("X", [BL, K, H], F32, kind="ExternalInput").ap()
    ht_d = nc.dram_tensor("h_t", [BL, T, H], F32, kind="ExternalInput").ap()
    Wx_d = nc.dram_tensor("Wx", [R, H], F32, kind="ExternalInput").ap()
    Wh_d = nc.dram_tensor("Wh", [R, H], F32, kind="ExternalInput").ap()
    Wa_d = nc.dram_tensor("Wa", [1, R], F32, kind="ExternalInput").ap()
    out_d = nc.dram_tensor("out", [BL, T, H], F32, kind="ExternalOutput").ap()

    with tile.TileContext(nc) as tc:
        with (
            tc.tile_pool(name="consts", bufs=1) as consts,
            tc.tile_pool(name="hin", bufs=3) as hin_pool,
            tc.tile_pool(name="xin", bufs=3) as xin_pool,
            tc.tile_pool(name="hbp", bufs=3) as hb_pool,
            tc.tile_pool(name="xbp", bufs=3) as xb_pool,
            tc.tile_pool(name="hTp", bufs=3) as hT_pool,
            tc.tile_pool(name="xTp", bufs=3) as xT_pool,
            tc.tile_pool(name="wk", bufs=3) as wk,
            tc.tile_pool(name="sm", bufs=3) as sm,
            tc.tile_pool(name="pcc", bufs=2, space="PSUM") as pcc,
            tc.tile_pool(name="ptp", bufs=2, space="PSUM") as ptp,
            tc.tile_pool(name="psZ", bufs=2, space="PSUM") as psZ,
            tc.tile_pool(name="psO", bufs=1, space="PSUM") as psO,
        ):
            # ---- identities for PE transposes ----
            ident = consts.tile([128, 128], F32)
            make_identity(nc, ident[:])
            identb = consts.tile([128, 128], BF16)
            make_identity(nc, identb[:])

            # ---- weights: load natural f32, PE-transpose, store bf16 ----
            def load_wt(w_dram, tag):
                wn = consts.tile([R, H], F32, tag="wnat")
                nc.sync.dma_start(out=wn[:], in_=_ap(w_dram, 0, [[H, R], [1, H]]))
                wt = consts.tile([128, HT * R], BF16, tag=tag)
                tp = ptp.tile([128, HT * R], F32, tag="tp")
                for j in range(HT):
                    nc.tensor.transpose(
                        tp[:, j * R : (j + 1) * R],
                        wn[:, j * 128 : (j + 1) * 128],
                        ident[0:R, 0:R],
                    )
                nc.vector.tensor_copy(wt[:], tp[:])
                return wt

            WhT = load_wt(Wh_d, "WhT")  # [128, 8*49] bf16; j-th tile at cols j*49
            WxT = load_wt(Wx_d, "WxT")

            # Wa2: [128, 2] block-diag: rows 0:49 col0 = Wa, rows 64:113 col1 = Wa
            Wa2f = consts.tile([128, 2], F32)
            nc.vector.memset(Wa2f[:], 0.0)
            nc.sync.dma_start(out=Wa2f[0:R, 0:1], in_=_ap(Wa_d, 0, [[1, R], [1, 1]]))
            nc.sync.dma_start(
                out=Wa2f[PB : PB + R, 1:2], in_=_ap(Wa_d, 0, [[1, R], [1, 1]])
            )
            Wa2 = consts.tile([128, 2], BF16)
            nc.vector.tensor_copy(Wa2[:], Wa2f[:])

            for b in range(BL):
                # ---- natural loads (f32) ----
                hn = hin_pool.tile([T, H], F32, tag="hn")
                nc.sync.dma_start(
                    out=hn[:], in_=_ap(ht_d, b * T * H, [[H, T], [1, H]])
                )
                xn = xin_pool.tile([K, H], F32, tag="xn")
                nc.sync.dma_start(
                    out=xn[0:K, :], in_=_ap(X_d, b * K * H, [[H, K], [1, H]])
                )

                # ---- bf16 conversion on DVE ----
                hb = hb_pool.tile([T, H], BF16, tag="hb")
                nc.scalar.copy(hb[:], hn[:])
                xb = xb_pool.tile([K, H], BF16, tag="xb")
                nc.vector.tensor_copy(xb[:], xn[0:K, :])

                # ---- PE transposes (bf16): hTb[p,j,t] = hb[t, j*128+p] ----
                hTb = hT_pool.tile([128, HT, T], BF16, tag="hTb")
                for rnd in range(2):
                    tp = ptp.tile([128, 512], BF16, tag="tp")
                    for jj in range(4):
                        j = 4 * rnd + jj
                        nc.tensor.transpose(
                            tp[:, jj * 128 : (jj + 1) * 128],
                            hb[:, j * 128 : (j + 1) * 128],
                            identb[:],
                        )
                    nc.vector.tensor_copy(hTb[:, 4 * rnd : 4 * rnd + 4, :], tp[:])
                KA = 50  # padded column stride (4-byte-aligned bf16 PSUM offsets)
                xTb = xT_pool.tile([128, HT, K], BF16, tag="xTb")
                tpx = ptp.tile([128, HT * KA], BF16, tag="tp")
                for j in range(HT):
                    nc.tensor.transpose(
                        tpx[:, j * KA : j * KA + K],
                        xb[:, j * 128 : (j + 1) * 128],
                        identb[0:K, 0:K],
                    )
                nc.vector.tensor_copy(
                    xTb[:], bass.AP(tensor=tpx[:].tensor, offset=tpx[:].offset,
                                    ap=[tpx[:].ap[0], [KA, HT], [1, K]])
                )

                # ---- chT = Wh @ h_t[b]^T : [49, 128]; cxT = Wx @ X^T : [49, 49] ----
                cc = pcc.tile([R, T + K], F32, tag="cc")
                chT = cc[:, 0:T]
                cxT = cc[:, T : T + K]
                for j in range(HT):
                    nc.tensor.matmul(
                        chT, WhT[:, j * R : (j + 1) * R], hTb[:, j, :],
                        start=(j == 0), stop=(j == HT - 1),
                    )
                for j in range(HT):
                    nc.tensor.matmul(
                        cxT, WxT[:, j * R : (j + 1) * R], xTb[:, j, :],
                        start=(j == 0), stop=(j == HT - 1),
                    )

                # replicate chT into partitions 0:49 and 64:113; zero the gap rows
                chT2 = wk.tile([128, T], BF16, tag="chT2")
                nc.vector.memset(chT2[32:PB, :], 0.0)
                nc.vector.tensor_copy(chT2[0:R, :], chT)
                nc.vector.tensor_copy(chT2[PB : PB + R, :], chT)

                # cxT2[128, 25]: rows 0:49 = even k columns, rows 64:113 = odd
                cxT2 = wk.tile([128, NPAIR], BF16, tag="cxT2")
                nc.vector.memset(cxT2[:], 0.0)
                st = cxT.ap[-1][0]
                nc.vector.tensor_copy(
                    cxT2[0:R, 0:NPAIR], _ap(cxT, 0, [cxT.ap[0], [2 * st, NPAIR]])
                )
                nc.vector.tensor_copy(
                    cxT2[PB : PB + R, 0 : NPAIR - 1],
                    _ap(cxT, st, [cxT.ap[0], [2 * st, NPAIR - 1]]),
                )

                # ---- S = tanh(chT2 (bcast over q) + cxT2 (bcast over t)), bf16 ----
                S = sm.tile([128, NPAIR, T], BF16, tag="S")
                c2 = chT2[:]
                x2 = cxT2[:]
                nc.vector.tensor_add(
                    S[0:PT, :, :],
                    _ap(c2, 0, [[c2.ap[0][0], PT], [0, NPAIR], c2.ap[-1]]),
                    _ap(x2, 0, [[x2.ap[0][0], PT], x2.ap[-1], [0, T]]),
                )
                nc.scalar.activation(
                    S[0:PT, :, :], S[0:PT, :, :], mybir.ActivationFunctionType.Tanh
                )

                # ---- z[t, k]: 25 bf16 matmuls, pair q -> columns (2q, 2q+1) ----
                zal = psZ.tile([T, 2 * NPAIR + T], F32, tag="z")
                z = zal[:, 0 : 2 * NPAIR]
                for q in range(NPAIR):
                    nc.tensor.matmul(
                        z[:, 2 * q : 2 * q + 2], S[0:PT, q, :], Wa2[0:PT, :],
                        start=True, stop=True,
                    )

                # ---- softmax over k (free axis), K=49 valid columns ----
                zmax = sm.tile([T, 1], F32, tag="zmax")
                nc.vector.reduce_max(zmax[:], z[:, 0:K], axis=mybir.AxisListType.X)
                zmaxn = sm.tile([T, 1], F32, tag="zmaxn")
                nc.vector.tensor_scalar_mul(zmaxn[:], zmax[:], -1.0)
                expz = sm.tile([T, K], F32, tag="expz")
                denom = sm.tile([T, 1], F32, tag="denom")
                nc.scalar.activation(
                    expz[:], z[:, 0:K], mybir.ActivationFunctionType.Exp,
                    bias=zmaxn[:], accum_out=denom[:],
                )
                rden = sm.tile([T, 1], F32, tag="rden")
                nc.vector.reciprocal(rden[:], denom[:])
                alpha = sm.tile([T, K], F32, tag="alpha")
                nc.vector.tensor_scalar_mul(alpha[:], expz[:], rden[:])

                # ---- alphaT via PE transpose (f32, into the z PSUM bank) ----
                alphaT_ps = zal[0:K, 2 * NPAIR : 2 * NPAIR + T]
                nc.tensor.transpose(alphaT_ps, alpha[:], ident[:])
                alphaT = sm.tile([K, T], BF16, tag="alphaT_sb")
                nc.vector.tensor_copy(alphaT[:], alphaT_ps)

                # ---- out[b] = alpha @ X[b] : [128, 1024] (bf16 x bf16 -> f32) ----
                ob = psO.tile([T, H], F32, tag="ob")
                for half in range(2):
                    nc.tensor.matmul(
                        ob[:, half * 512 : (half + 1) * 512],
                        alphaT[:],
                        xb[:, half * 512 : (half + 1) * 512],
                        start=True, stop=True,
                    )
                osb = sm.tile([T, H], F32, tag="osb")
                nc.vector.tensor_copy(osb[:, 0:512], ob[:, 0:512])
                nc.scalar.copy(osb[:, 512:1024], ob[:, 512:1024])
                nc.sync.dma_start(
                    out=_ap(out_d, b * T * H, [[H, T], [1, H]]), in_=osb[:]
                )

    nc.compile()
    return nc


def _get_nc():
    if "nc" not in _CACHE:
        _CACHE["nc"] = build()
    return _CACHE["nc"]


def kernel(X, h_t, Wx, Wh, Wa):
    nc = _get_nc()
    X = np.ascontiguousarray(X, dtype=np.float32)
    h_t = np.ascontiguousarray(h_t, dtype=np.float32)
    Wx = np.ascontiguousarray(Wx, dtype=np.float32)
    Wh = np.ascontiguousarray(Wh, dtype=np.float32)
    Wa = np.ascontiguousarray(Wa, dtype=np.float32)
    in_maps = [
        {
            "X": X[c * BL : (c + 1) * BL],
            "h_t": h_t[c * BL : (c + 1) * BL],
            "Wx": Wx,
            "Wh": Wh,
            "Wa": Wa,
        }
        for c in range(NCORES)
    ]
    res = run_bass_kernel_spmd(nc, in_maps, core_ids=list(range(NCORES)))
    return np.concatenate([res.results[c]["out"] for c in range(NCORES)], axis=0)


# revision 15
# speedup vs baseline: 2.4134x; 1.0026x over previous
"""Trainium2 Bass kernel for nn_AttentionBlock: 8-core data-parallel over batch.

Reference computation (per batch b):
  cx = X[b] @ Wx^T               [K,R]   (K=49 regions, R=49, H=1024)
  ch = h_t[b] @ Wh^T             [T,R]   (T=128)
  z[t,k] = sum_r Wa[r] * tanh(cx[k,r] + ch[t,r])
  alpha = softmax_k(z)           [T,K]
  out[b] = alpha @ X[b]          [T,H]

Sharding: data-parallel across batch B=128 on 8 cores (16 batches each);
weights replicated. No collectives.

v2 layout strategy per batch (all matmuls in bf16, fp32 PSUM accum):
  - load h_t[b]/X[b] naturally (f32), convert to bf16 on GpSimd,
    transpose via xbar DMA (SBUF->SBUF, 2-byte dtype) -> hTb [h,j,t], xTb [h,j,k]
  - chT[r,t] / cxT[r,k] via bf16 matmuls contracting h (WT stationary)
  - pack r twice (partitions 0:49 and 64:113): S[113, 25, 128] =
    tanh(chT2 + cxT2) via DVE broadcast add + one big ScalarE tanh
  - z[t, 2q:2q+2] = S[:,q,:].T @ Wa2 (block-diag Wa, zero rows kill the gap)
  - free-axis softmax with fused exp+accum; alpha^T via PE transpose
  - out[b] = alphaT.T @ X[b] (bf16); copy PSUM->SBUF; DMA out (f32)
"""

import sys

sys.path.insert(0, "/opt/trn_rl_repo")

import numpy as np

import concourse.bass as bass
import concourse.bacc as bacc
import concourse.tile as tile
from concourse import mybir
from concourse.bass_utils import run_bass_kernel_spmd
from concourse.masks import make_identity

B, T, K, H = 128, 128, 49, 1024
R = 49
NCORES = 8
BL = B // NCORES  # batches per core
HT = H // 128  # h tiles
NPAIR = (K + 1) // 2  # 25 k-pairs (last pair half-garbage, ignored)
PB = 64  # partition offset of the second r-block (must be mult of 32)
PT = PB + R  # 113 partitions used by the packed S / Wa2
KP = 64  # X partition count padded for xbar transpose (needs mult of 16)
F32 = mybir.dt.float32
BF16 = mybir.dt.bfloat16

_CACHE = {}


def _ap(base, off, dims):
    """Custom access pattern on the tensor underlying `base` (an AP)."""
    return bass.AP(tensor=base.tensor, offset=base.offset + off, ap=dims)


def build():
    nc = bacc.Bacc("TRN2", target_bir_lowering=False, debug=False, num_devices=NCORES)

    X_d = nc.dram_tensor("X", [BL, K, H], F32, kind="ExternalInput").ap()
    ht_d = nc.dram_tensor("h_t", [BL, T, H], F32, kind="ExternalInput").ap()
    Wx_d = nc.dram_tensor("Wx", [R, H], F32, kind="ExternalInput").ap()
    Wh_d = nc.dram_tensor("Wh", [R, H], F32, kind="ExternalInput").ap()
    Wa_d = nc.dram_tensor("Wa", [1, R], F32, kind="ExternalInput").ap()
    out_d = nc.dram_tensor("out", [BL, T, H], F32, kind="ExternalOutput").ap()

    with tile.TileContext(nc) as tc:
        with (
            tc.tile_pool(name="consts", bufs=1) as consts,
            tc.tile_pool(name="hin", bufs=3) as hin_pool,
            tc.tile_pool(name="xin", bufs=3) as xin_pool,
            tc.tile_pool(name="hbp", bufs=3) as hb_pool,
            tc.tile_pool(name="xbp", bufs=3) as xb_pool,
            tc.tile_pool(name="hTp", bufs=3) as hT_pool,
            tc.tile_pool(name="xTp", bufs=3) as xT_pool,
            tc.tile_pool(name="wk", bufs=3) as wk,
            tc.tile_pool(name="sm", bufs=3) as sm,
            tc.tile_pool(name="pcc", bufs=2, space="PSUM") as pcc,
            tc.tile_pool(name="ptp", bufs=2, space="PSUM") as ptp,
            tc.tile_pool(name="psZ", bufs=2, space="PSUM") as psZ,
            tc.tile_pool(name="psO", bufs=1, space="PSUM") as psO,
        ):
            # ---- identities for PE transposes ----
            ident = consts.tile([128, 128], F32)
            make_identity(nc, ident[:])
            identb = consts.tile([128, 128], BF16)
            make_identity(nc, identb[:])

            # ---- weights: load natural f32, PE-transpose, store bf16 ----
            def load_wt(w_dram, tag):
                wn = consts.tile([R, H], F32, tag="wnat")
                nc.sync.dma_start(out=wn[:], in_=_ap(w_dram, 0, [[H, R], [1, H]]))
                wt = consts.tile([128, HT * R], BF16, tag=tag)
                tp = ptp.tile([128, HT * R], F32, tag="tp")
                for j in range(HT):
                    nc.tensor.transpose(
                        tp[:, j * R : (j + 1) * R],
                        wn[:, j * 128 : (j + 1) * 128],
                        ident[0:R, 0:R],
                    )
                nc.vector.tensor_copy(wt[:], tp[:])
                return wt

            WhT = load_wt(Wh_d, "WhT")  # [128, 8*49] bf16; j-th tile at cols j*49
            WxT = load_wt(Wx_d, "WxT")

            # Wa2: [128, 2] block-diag: rows 0:49 col0 = Wa, rows 64:113 col1 = Wa
            Wa2f = consts.tile([128, 2], F32)
            nc.vector.memset(Wa2f[:], 0.0)
            nc.sync.dma_start(out=Wa2f[0:R, 0:1], in_=_ap(Wa_d, 0, [[1, R], [1, 1]]))
            nc.sync.dma_start(
                out=Wa2f[PB : PB + R, 1:2], in_=_ap(Wa_d, 0, [[1, R], [1, 1]])
            )
            Wa2 = consts.tile([128, 2], BF16)
            nc.vector.tensor_copy(Wa2[:], Wa2f[:])

            for b in range(BL):
                # ---- natural loads (f32) ----
                hn = hin_pool.tile([T, H], F32, tag="hn")
                nc.sync.dma_start(
                    out=hn[:], in_=_ap(ht_d, b * T * H, [[H, T], [1, H]])
                )
                xn = xin_pool.tile([K, H], F32, tag="xn")
                nc.sync.dma_start(
                    out=xn[0:K, :], in_=_ap(X_d, b * K * H, [[H, K], [1, H]])
                )

                # ---- bf16 conversion on DVE ----
                hb = hb_pool.tile([T, H], BF16, tag="hb")
                nc.scalar.copy(hb[:], hn[:])
                xb = xb_pool.tile([K, H], BF16, tag="xb")
                nc.scalar.copy(xb[:], xn[0:K, :])

                # ---- PE transposes (bf16): hTb[p,j,t] = hb[t, j*128+p] ----
                hTb = hT_pool.tile([128, HT, T], BF16, tag="hTb")
                for rnd in range(2):
                    tp = ptp.tile([128, 512], BF16, tag="tp")
                    for jj in range(4):
                        j = 4 * rnd + jj
                        nc.tensor.transpose(
                            tp[:, jj * 128 : (jj + 1) * 128],
                            hb[:, j * 128 : (j + 1) * 128],
                            identb[:],
                        )
                    nc.vector.tensor_copy(hTb[:, 4 * rnd : 4 * rnd + 4, :], tp[:])
                KA = 50  # padded column stride (4-byte-aligned bf16 PSUM offsets)
                xTb = xT_pool.tile([128, HT, K], BF16, tag="xTb")
                tpx = ptp.tile([128, HT * KA], BF16, tag="tp")
                for j in range(HT):
                    nc.tensor.transpose(
                        tpx[:, j * KA : j * KA + K],
                        xb[:, j * 128 : (j + 1) * 128],
                        identb[0:K, 0:K],
                    )
                nc.vector.tensor_copy(
                    xTb[:], bass.AP(tensor=tpx[:].tensor, offset=tpx[:].offset,
                                    ap=[tpx[:].ap[0], [KA, HT], [1, K]])
                )

                # ---- chT = Wh @ h_t[b]^T : [49, 128]; cxT = Wx @ X^T : [49, 49] ----
                cc = pcc.tile([R, T + K], F32, tag="cc")
                chT = cc[:, 0:T]
                cxT = cc[:, T : T + K]
                for j in range(HT):
                    nc.tensor.matmul(
                        chT, WhT[:, j * R : (j + 1) * R], hTb[:, j, :],
                        start=(j == 0), stop=(j == HT - 1),
                    )
                for j in range(HT):
                    nc.tensor.matmul(
                        cxT, WxT[:, j * R : (j + 1) * R], xTb[:, j, :],
                        start=(j == 0), stop=(j == HT - 1),
                    )

                # replicate chT into partitions 0:49 and 64:113; zero the gap rows
                chT2 = wk.tile([128, T], BF16, tag="chT2")
                if b < 3:
                    nc.vector.memset(chT2[32:PB, :], 0.0)
                nc.vector.tensor_copy(chT2[0:R, :], chT)
                nc.vector.tensor_copy(chT2[PB : PB + R, :], chT)

                # cxT2[128, 25]: rows 0:49 = even k columns, rows 64:113 = odd
                cxT2 = wk.tile([128, NPAIR], BF16, tag="cxT2")
                if b < 3:
                    nc.vector.memset(cxT2[:], 0.0)
                st = cxT.ap[-1][0]
                nc.vector.tensor_copy(
                    cxT2[0:R, 0:NPAIR], _ap(cxT, 0, [cxT.ap[0], [2 * st, NPAIR]])
                )
                nc.vector.tensor_copy(
                    cxT2[PB : PB + R, 0 : NPAIR - 1],
                    _ap(cxT, st, [cxT.ap[0], [2 * st, NPAIR - 1]]),
                )

                # ---- S = tanh(chT2 (bcast over q) + cxT2 (bcast over t)), bf16 ----
                S = sm.tile([128, NPAIR, T], BF16, tag="S")
                c2 = chT2[:]
                x2 = cxT2[:]
                nc.vector.tensor_add(
                    S[0:PT, :, :],
                    _ap(c2, 0, [[c2.ap[0][0], PT], [0, NPAIR], c2.ap[-1]]),
                    _ap(x2, 0, [[x2.ap[0][0], PT], x2.ap[-1], [0, T]]),
                )
                nc.scalar.activation(
                    S[0:PT, :, :], S[0:PT, :, :], mybir.ActivationFunctionType.Tanh
                )

                # ---- z[t, k]: 25 bf16 matmuls, pair q -> columns (2q, 2q+1) ----
                zal = psZ.tile([T, 2 * NPAIR + T], F32, tag="z")
                z = zal[:, 0 : 2 * NPAIR]
                for q in range(NPAIR):
                    nc.tensor.matmul(
                        z[:, 2 * q : 2 * q + 2], S[0:PT, q, :], Wa2[0:PT, :],
                        start=True, stop=True,
                    )

                # ---- softmax over k (free axis), K=49 valid columns ----
                zmax = sm.tile([T, 1], F32, tag="zmax")
                nc.vector.reduce_max(zmax[:], z[:, 0:K], axis=mybir.AxisListType.X)
                zmaxn = sm.tile([T, 1], F32, tag="zmaxn")
                nc.vector.tensor_scalar_mul(zmaxn[:], zmax[:], -1.0)
                expz = sm.tile([T, K], F32, tag="expz")
                denom = sm.tile([T, 1], F32, tag="denom")
                nc.scalar.activation(
                    expz[:], z[:, 0:K], mybir.ActivationFunctionType.Exp,
                    bias=zmaxn[:], accum_out=denom[:],
                )
                rden = sm.tile([T, 1], F32, tag="rden")
                nc.vector.reciprocal(rden[:], denom[:])

                # ---- alphaT via PE transpose (unnormalized; 1/denom folded
                #      into the output copy) ----
                alphaT_ps = zal[0:K, 2 * NPAIR : 2 * NPAIR + T]
                nc.tensor.transpose(alphaT_ps, expz[:], ident[:])
                alphaT = sm.tile([K, T], BF16, tag="alphaT_sb")
                nc.vector.tensor_copy(alphaT[:], alphaT_ps)

                # ---- out[b] = alpha @ X[b] : [128, 1024] (bf16 x bf16 -> f32) ----
                ob = psO.tile([T, H], F32, tag="ob")
                for half in range(2):
                    nc.tensor.matmul(
                        ob[:, half * 512 : (half + 1) * 512],
                        alphaT[:],
                        xb[:, half * 512 : (half + 1) * 512],
                        start=True, stop=True,
                    )
                osb = sm.tile([T, H], F32, tag="osb")
                nc.vector.tensor_scalar_mul(osb[:, 0:512], ob[:, 0:512], rden[:])
                nc.scalar.activation(
                    osb[:, 512:1024], ob[:, 512:1024],
                    mybir.ActivationFunctionType.Copy, scale=rden[:],
                )
                nc.sync.dma_start(
                    out=_ap(out_d, b * T * H, [[H, T], [1, H]]), in_=osb[:]
                )

    nc.compile()
    return nc


def _get_nc():
    if "nc" not in _CACHE:
        _CACHE["nc"] = build()
    return _CACHE["nc"]


def kernel(X, h_t, Wx, Wh, Wa):
    nc = _get_nc()
    X = np.ascontiguousarray(X, dtype=np.float32)
    h_t = np.ascontiguousarray(h_t, dtype=np.float32)
    Wx = np.ascontiguousarray(Wx, dtype=np.float32)
    Wh = np.ascontiguousarray(Wh, dtype=np.float32)
    Wa = np.ascontiguousarray(Wa, dtype=np.float32)
    in_maps = [
        {
            "X": X[c * BL : (c + 1) * BL],
            "h_t": h_t[c * BL : (c + 1) * BL],
            "Wx": Wx,
            "Wh": Wh,
            "Wa": Wa,
        }
        for c in range(NCORES)
    ]
    res = run_bass_kernel_spmd(nc, in_maps, core_ids=list(range(NCORES)))
    return np.concatenate([res.results[c]["out"] for c in range(NCORES)], axis=0)
